# revision 1
# baseline (speedup 1.0000x reference)
"""Distributed causal multi-head attention kernel for 8 TRN2 NeuronCores.

Problem: B=2, S=2048, D=1024, H=16 heads (hd=64), f32 I/O, causal softmax.
Sharding: data-parallel over batch (2 groups of 4 cores), tensor-parallel over
heads within each group (4 heads/core) and over wo output columns.

Per-core compute (all transposed layouts, bf16 matmul, f32 PSUM accum):
  Q^T,K^T = wqT/wkT_shard.T @ x^T      [256, 2048]
  V       = x @ wv_shard.T             [2048, 4x(64+1)]  (ones col for denom)
  S^T     = K_h @ Q_h^T (per head, row-paired on the PE array)
  P~      = exp(S^T/8) * causal_mask   (ACT exp, DVE mask mul)
  Y_h^T,den = [V_h|1].T @ P~           (AV matmul, row 64 = softmax denom)
  yT      = Y_h^T / den
  AllGather yT over the 4-core group (chunked by q so comm overlaps compute)
  F^T_shard = woT_shard.T @ yT_full    [256, 2048] -> output

"""
import numpy as np
import ml_dtypes

import concourse.bass as bass
import concourse.bacc as bacc
import concourse.tile as tile
from concourse import mybir
from concourse.bass import ts, _add_dep_helper

B, S, D, H = 2, 2048, 1024, 16
HD = D // H            # 64
N_CORES = 8
TP = 4                 # cores per batch group
HPC = H // TP          # heads per core = 4
DPC = D // TP          # 256: head-dims per core, also wo dout shard
GROUPS = [[0, 1, 2, 3], [4, 5, 6, 7]]
QC = 512               # q-chunk (free dim of scores)
KT = 128               # k-tile (partition dim of scores)
NQC = S // QC          # 4
NKT = S // KT          # 16
NK = D // 128          # 8 contraction tiles for projections

BF16 = mybir.dt.bfloat16
F32 = mybir.dt.float32


def build():
    nc = bacc.Bacc(None, target_bir_lowering=False, debug=False)

    xT = nc.declare_dram_parameter("xT", [D, S], BF16, isOutput=False)
    wqT = nc.declare_dram_parameter("wqT", [D, DPC], BF16, isOutput=False)
    wkT = nc.declare_dram_parameter("wkT", [D, DPC], BF16, isOutput=False)
    wvT = nc.declare_dram_parameter("wvT", [D, DPC], BF16, isOutput=False)
    woT = nc.declare_dram_parameter("woT", [D, DPC], BF16, isOutput=False)
    masks = nc.declare_dram_parameter("masks", [4, KT, 2 * QC], BF16, isOutput=False)
    out = nc.declare_dram_parameter("out", [DPC, S], F32, isOutput=True)

    with tile.TileContext(nc) as tc:
        with (
            tc.tile_pool(name="persist", bufs=1) as persist,
            tc.tile_pool(name="xtp", bufs=1) as xtp,
            tc.tile_pool(name="ptile", bufs=4) as ptile,
            tc.tile_pool(name="norm", bufs=6) as norm,
            tc.tile_pool(name="gq", bufs=1) as gqp,
            tc.tile_pool(name="dram", bufs=1, space="DRAM") as dram,
        ):
            # ---- persistent SBUF ----
            xt = [xtp.tile([128, S], BF16, name=f"xt{k}") for k in range(NK)]
            wq_s3 = persist.tile([128, NK, DPC], BF16, name="wq_s3")
            wk_s3 = persist.tile([128, NK, DPC], BF16, name="wk_s3")
            wv_s3 = persist.tile([128, NK, DPC], BF16, name="wv_s3")
            wo_s3 = persist.tile([128, NK, DPC], BF16, name="wo_s3")
            wq_s = [wq_s3[:, k, :] for k in range(NK)]
            wk_s = [wk_s3[:, k, :] for k in range(NK)]
            wv_s = [wv_s3[:, k, :] for k in range(NK)]
            wo_s = [wo_s3[:, k, :] for k in range(NK)]
            mask_t = [persist.tile([KT, 2 * QC], BF16, name=f"mask{r}") for r in range(4)]
            qT = [persist.tile([128, S], BF16, name=f"qT{hp}") for hp in range(2)]
            kT = [persist.tile([128, S], BF16, name=f"kT{hp}") for hp in range(2)]
            vt = [persist.tile([128, HPC, HD + 1], BF16, name=f"v{st}") for st in range(NKT)]
            yT = [persist.tile([HD, S], BF16, name=f"yT{h}") for h in range(HPC)]

            # first compute needs wq + wk + xt[nq=0]; order DMAs accordingly
            nc.gpsimd.dma_start(
                out=wq_s3, in_=wqT.rearrange("(k p) d -> p k d", p=128)
            )
            nc.gpsimd.dma_start(
                out=wk_s3, in_=wkT.rearrange("(k p) d -> p k d", p=128)
            )
            for k in range(NK):
                nc.sync.dma_start(out=xt[k][:, ts(0, QC)], in_=xT[ts(k, 128), ts(0, QC)])
            nc.gpsimd.dma_start(
                out=wv_s3, in_=wvT.rearrange("(k p) d -> p k d", p=128)
            )
            for r in range(4):
                nc.gpsimd.dma_start(out=mask_t[r], in_=masks[r])
            for nq in range(1, NQC):
                for k in range(NK):
                    nc.sync.dma_start(
                        out=xt[k][:, ts(nq, QC)], in_=xT[ts(k, 128), ts(nq, QC)]
                    )
            nc.gpsimd.dma_start(
                out=wo_s3, in_=woT.rearrange("(k p) d -> p k d", p=128)
            )

            # warm up the ACT exp table set during the DMA preamble so the
            # first real exp doesn't pay the ~2.7us ACT_TABLE_LOAD
            warm = persist.tile([128, 16], F32, name="warm")
            nc.vector.memset(warm, 0.0)
            warm2 = persist.tile([128, 16], F32, name="warm2")
            nc.scalar.activation(
                warm2, warm, mybir.ActivationFunctionType.Exp, scale=1.0
            )

            # tiny dummy AllGather during the preamble: synchronizes the
            # cores while they are all loading inputs, so the real AGs
            # later don't absorb inter-core startup skew
            sync_in = dram.tile([1, 16], BF16, name="sync_in")
            sync_sb = persist.tile([1, 16], BF16, name="sync_sb")
            nc.vector.memset(sync_sb, 0.0)
            nc.sync.dma_start(out=sync_in, in_=sync_sb)
            sync_out = dram.tile([4, 16], BF16, name="sync_out")
            nc.gpsimd.collective_compute(
                "AllGather",
                mybir.AluOpType.bypass,
                replica_groups=GROUPS,
                ins=[sync_in[:].opt()],
                outs=[sync_out[:].opt()],
            )

            gath = []
            gqs = []
            last_bcast = [None]

            def proj_chunk(nq):
                for m in range(2):  # dout 128-tiles of the 256 local head dims
                    qp = pj_ps.tile([128, QC], F32, tag="qp", name=f"qp{nq}_{m}")
                    for k in range(NK):
                        nc.tensor.matmul(
                            qp, wq_s[k][:, ts(m, 128)], xt[k][:, ts(nq, QC)],
                            start=(k == 0), stop=(k == NK - 1),
                        )
                    nc.vector.tensor_copy(qT[m][:, ts(nq, QC)], qp)
                    kp = pj_ps.tile([128, QC], F32, tag="kp", name=f"kp{nq}_{m}")
                    for k in range(NK):
                        nc.tensor.matmul(
                            kp, wk_s[k][:, ts(m, 128)], xt[k][:, ts(nq, QC)],
                            start=(k == 0), stop=(k == NK - 1),
                        )
                    nc.vector.tensor_copy(kT[m][:, ts(nq, QC)], kp)
                for sm in range(4):  # s-tiles of 128 inside this q-chunk
                    st = nq * 4 + sm
                    vp = pj_ps.tile([128, DPC], F32, tag="vp", name=f"vp{st}")
                    for k in range(NK):
                        nc.tensor.matmul(
                            vp, xt[k][:, ts(st, 128)], wv_s[k],
                            start=(k == 0), stop=(k == NK - 1),
                        )
                    nc.vector.memset(vt[st][:, :, HD:HD + 1], 1.0)
                    nc.vector.tensor_copy(
                        vt[st][:, :, 0:HD],
                        vp[:].rearrange("p (h d) -> p h d", h=HPC),
                    )

            def attn_chunk(qc):
                n_k = (qc + 1) * 4
                for hp in range(2):
                    avA = av_ps.tile([128, QC], F32, tag="avA", name=f"avA{qc}_{hp}")
                    avB = av_ps.tile([128, QC], F32, tag="avB", name=f"avB{qc}_{hp}")
                    for m in range(n_k):
                        # causal: columns j < off are fully masked for this k-tile
                        off = max(0, (m - 4 * qc) * 128)
                        sc = sc_ps.tile([128, 2 * QC], F32, tag="sc", name=f"sc{qc}_{hp}_{m}")
                        sc3 = sc[:].rearrange("p (t q) -> p t q", t=2)
                        nc.tensor.matmul(
                            sc[:, off:QC],
                            kT[hp][0:64, ts(m, 128)],
                            qT[hp][0:64, qc * QC + off:(qc + 1) * QC],
                            start=True, stop=True,
                        )
                        nc.tensor.matmul(
                            sc[:, QC + off:2 * QC],
                            kT[hp][64:128, ts(m, 128)],
                            qT[hp][64:128, qc * QC + off:(qc + 1) * QC],
                            start=True, stop=True,
                        )
                        pt = ptile.tile([128, 2 * QC], BF16, tag="pt", name=f"pt{qc}_{hp}_{m}")
                        pt3 = pt[:].rearrange("p (t q) -> p t q", t=2)
                        nc.scalar.activation(
                            pt3[:, :, off:QC], sc3[:, :, off:QC],
                            mybir.ActivationFunctionType.Exp,
                            scale=1.0 / np.sqrt(HD),
                        )
                        if m >= 4 * qc:
                            ri = m - 4 * qc
                            m3 = mask_t[ri][:].rearrange("p (t q) -> p t q", t=2)
                            nc.vector.tensor_mul(
                                pt3[:, :, off:QC], pt3[:, :, off:QC],
                                m3[:, :, off:QC],
                            )
                        nc.tensor.matmul(
                            avA[0:HD + 1, off:QC], vt[m][:, 2 * hp, :],
                            pt[:, off:QC],
                            start=(m == 0), stop=(m == n_k - 1),
                        )
                        nc.tensor.matmul(
                            avB[0:HD + 1, off:QC], vt[m][:, 2 * hp + 1, :],
                            pt[:, QC + off:2 * QC],
                            start=(m == 0), stop=(m == n_k - 1),
                        )
                    for hh, av in ((2 * hp, avA), (2 * hp + 1, avB)):
                        dn = norm.tile([1, QC], F32, tag="dn", name=f"dn{qc}_{hh}")
                        nc.vector.tensor_copy(dn, av[HD:HD + 1, :])
                        recip = norm.tile([1, QC], F32, tag="recip", name=f"rc{qc}_{hh}")
                        nc.vector.reciprocal_approx_fast(recip, dn)
                        rb = norm.tile([HD, QC], F32, tag="rb", name=f"rb{qc}_{hh}")
                        bc = nc.gpsimd.partition_broadcast(rb, recip, channels=HD)
                        last_bcast[0] = bc.ins
                        nc.vector.tensor_mul(yT[hh][:, ts(qc, QC)], av[0:HD, :], rb)

                # after every second q-chunk, ship that half of yT through
                # the group AllGather (few big collectives beat many small)
                if qc % 2 == 1:
                    half = qc // 2
                    bounce = dram.tile([DPC, 2 * QC], BF16, name=f"bounce{half}")
                    for h in range(HPC):
                        nc.sync.dma_start(
                            out=bounce[ts(h, HD), :],
                            in_=yT[h][:, half * 2 * QC:(half + 1) * 2 * QC],
                        )
                    g = dram.tile([D, 2 * QC], BF16, name=f"gath{half}")
                    nc.gpsimd.collective_compute(
                        "AllGather",
                        mybir.AluOpType.bypass,
                        replica_groups=GROUPS,
                        ins=[bounce[:].opt()],
                        outs=[g[:].opt()],
                    )
                    gath.append(g)

            # phase 1: all projections; phase 2: attention (separate PSUM pools)
            pj_ctx = tc.tile_pool(name="pj_ps", bufs=2, space="PSUM")
            pj_ps = pj_ctx.__enter__()
            for nq in range(NQC):
                proj_chunk(nq)
            pj_ctx.__exit__(None, None, None)
            sc_ctx = tc.tile_pool(name="sc_ps", bufs=2, space="PSUM")
            sc_ps = sc_ctx.__enter__()
            av_ctx = tc.tile_pool(name="av_ps", bufs=2, space="PSUM")
            av_ps = av_ctx.__enter__()
            for qc in range(NQC):
                attn_chunk(qc)
            av_ctx.__exit__(None, None, None)
            sc_ctx.__exit__(None, None, None)

            # gathered-yT loads after all attention so their AG waits don't
            # head-block the gpsimd queue (which also runs the broadcasts)
            for half in range(2):
                gq = [
                    gqp.tile([128, 2 * QC], BF16, tag=f"gq{half}_{k}", name=f"gq{half}_{k}")
                    for k in range(NK)
                ]
                for k in range(NK):
                    ld = nc.gpsimd.dma_start(out=gq[k], in_=gath[half][ts(k, 128), :])
                    # keep these off the gpsimd queue until all normalize
                    # broadcasts are done: their AG wait must not head-block
                    # the attention pipeline
                    _add_dep_helper(
                        ld.ins, last_bcast[0], sync=False,
                        reason="gq loads after attention broadcasts",
                    )
                gqs.append(gq)

            # ---- wo tail: F^T[dout_shard, :] = woT_shard.T @ yT_full ----
            wo_ctx = tc.tile_pool(name="wo_ps", bufs=2, space="PSUM")
            wo_ps = wo_ctx.__enter__()
            for half in range(2):
                for m in range(2):
                    wp = wo_ps.tile([128, 2 * QC], F32, tag="wp", name=f"wp{half}_{m}")
                    for q2 in range(2):
                        for k in range(NK):
                            nc.tensor.matmul(
                                wp[:, ts(q2, QC)], wo_s[k][:, ts(m, 128)],
                                gqs[half][k][:, ts(q2, QC)],
                                start=(k == 0), stop=(k == NK - 1),
                            )
                        ow = norm.tile(
                            [128, QC], F32, tag="ow", name=f"ow{half}_{m}_{q2}"
                        )
                        nc.vector.tensor_copy(ow, wp[:, ts(q2, QC)])
                        nc.sync.dma_start(
                            out=out[ts(m, 128), (2 * half + q2) * QC:(2 * half + q2 + 1) * QC],
                            in_=ow,
                        )
            wo_ctx.__exit__(None, None, None)

    nc.finalize()
    return nc


def make_masks():
    i = np.arange(KT)[:, None]
    j = np.arange(QC)[None, :]
    m = np.zeros((4, KT, 2 * QC), dtype=ml_dtypes.bfloat16)
    for r in range(4):
        half = ((r * KT + i) <= j).astype(ml_dtypes.bfloat16)
        m[r, :, 0:QC] = half
        m[r, :, QC:2 * QC] = half
    return m


def shard_inputs(x, wq, wk, wv, wo):
    """Full f32 inputs -> per-core in_maps (bf16)."""
    bf = ml_dtypes.bfloat16
    masks = make_masks()
    wqT = np.ascontiguousarray(wq.T).astype(bf)
    wkT = np.ascontiguousarray(wk.T).astype(bf)
    wvT = np.ascontiguousarray(wv.T).astype(bf)
    woT = np.ascontiguousarray(wo.T).astype(bf)
    in_maps = []
    for c in range(N_CORES):
        b, tp = divmod(c, TP)
        sl = slice(tp * DPC, (tp + 1) * DPC)
        in_maps.append({
            "xT": np.ascontiguousarray(x[b].T).astype(bf),
            "wqT": np.ascontiguousarray(wqT[:, sl]),
            "wkT": np.ascontiguousarray(wkT[:, sl]),
            "wvT": np.ascontiguousarray(wvT[:, sl]),
            "woT": np.ascontiguousarray(woT[:, sl]),
            "masks": masks,
        })
    return in_maps


def assemble_output(results):
    """Per-core F^T shards [DPC, S] -> full [B, S, D] f32."""
    outs = []
    for b in range(B):
        ft = np.concatenate(
            [results[b * TP + tp]["out"] for tp in range(TP)], axis=0
        )  # [D, S]
        outs.append(ft.T)  # [S, D]
    return np.stack(outs, axis=0)


_NC_CACHE = []


def kernel(x, wq, wk, wv, wo):
    """Full-input distributed attention on 8 NeuronCores; returns full output."""
    x = np.asarray(x, dtype=np.float32)
    wq = np.asarray(wq, dtype=np.float32)
    wk = np.asarray(wk, dtype=np.float32)
    wv = np.asarray(wv, dtype=np.float32)
    wo = np.asarray(wo, dtype=np.float32)
    if not _NC_CACHE:
        _NC_CACHE.append(build())
    nc = _NC_CACHE[0]
    in_maps = shard_inputs(x, wq, wk, wv, wo)
    from concourse import bass2jax
    results = bass2jax.run_bass_via_pjrt(nc, in_maps, n_cores=N_CORES)
    return assemble_output(results).astype(np.float32)

